# revision 1
# baseline (speedup 1.0000x reference)
"""NT-Xent contrastive loss (SimCLR) on 8 Trainium2 NeuronCores.

Strategy (data-parallel, fully SPMD — no collectives):
  - z = concat(z_i, z_j) [8192, 1024], cast bf16 on host.
  - Each core c gets a ROTATED, transposed view of z (rows rolled by
    -c*1024): zt = z_rot.T [1024, 8192], kept fully resident in SBUF. The
    rotation puts each core's own 1024 rows at index 0, so the
    self-diagonal / positive-pair positions are the same compile-time
    constants on every core -> one NEFF for all 8 cores.
  - Norms are computed from the resident zt: DVE squares each k-tile
    chunk, PE reduces over the partition (k) axis with ones-matmuls whose
    STATIONARY operand is the squared chunk (output lands partition-major
    [128, 64]).  inv = 1/max(sqrt(sumsq), eps); a PE transpose + DRAM
    bounce + partition-broadcast DMA yields the column-wise table Bt.
  - Main loop over 64 [128 rows x 1024 cols] tiles: G = Z_own @ Z.T on PE
    (bf16, f32 psum accum); S = G * inv_r * inv_c in one DVE
    scalar_tensor_tensor into SBUF; self-diagonal masked with -100;
    exp(S/T) with fused per-row accumulation on ACT; positives extracted
    with an identity-masked multiply + fused accum; lse = ln(sumexp);
    partial = sum_rows(lse - pos) reduced to a scalar with a ones-matmul.
  - Host sums the 8 per-core partials and divides by 2N.

This container's walrus build only accepts ONE semaphore wait per
instruction (and none on CTRL-encoded ones like Drain), while Tile freely
emits several. Two workarounds below: the TileContext epilogue drain's waits
are re-emitted on DVE memsets, and a post-pass splits any multi-wait
instruction by inserting single-wait no-op "carrier" clones (per-engine
templates) just before it on the same engine stream.
"""

import copy

import numpy as np
import ml_dtypes


def _install_tile_drain_patch():
    import concourse.tile as tile
    from concourse import mybir
    from concourse.vector_clock import ScopedClock

    if getattr(tile.TileContext, "_drain_patch_installed", False):
        return

    def _drain_and_barrier(self, tick_clock, wait_clock):
        nc = self.nc
        drain_inst = nc.sync.drain()
        wait_clock.add_sem_waits(
            drain_inst.ins, ScopedClock({None: tick_clock.global_clock})
        )
        waits = list(drain_inst.ins.sync_info.on_wait)
        drain_inst.ins.sync_info.on_wait.clear()

        if waits:
            scr = nc.const_aps.tensor(0.0, (1, 1), mybir.dt.float32)
            for w in waits:
                ms = nc.vector.memset(scr, 0)
                if ms.ins.sync_info is None:
                    ms.ins.sync_info = mybir.SyncInfo(on_wait=[], on_update=[])
                ms.ins.sync_info.on_wait.append(w)

        nc.all_engine_barrier()
        assert self.sems is not None
        popped = nc._tile_sem_poison_stack.pop()
        assert popped is self._sem_poison
        nc.clear_and_free_semaphores(list(self.sems.allocated().values()))
        nc.all_engine_barrier()

    tile.TileContext._drain_and_barrier = _drain_and_barrier
    tile.TileContext._drain_patch_installed = True


_install_tile_drain_patch()

import concourse.bass as bass
import concourse.tile as tile
from concourse import mybir
from concourse.bass_utils import run_bass_kernel_spmd
from concourse.masks import make_identity

P = 128
D = 1024
R = 8192          # 2N rows
MY = 1024         # rows per core
KT = D // P       # 8 k-tiles
MT = MY // P      # 8 m-tiles
CW = 1024         # column chunk width
NCH = R // CW     # 8 chunks
CB = R // P       # 64 column blocks (norm layout)
TEMP = 0.07
BF16 = mybir.dt.bfloat16
F32 = mybir.dt.float32
AX = mybir.AxisListType
ALU = mybir.AluOpType
ACTF = mybir.ActivationFunctionType

TRACE = False          # set True externally (test harness) for NTFF profiling
LAST_RESULTS = None    # BassKernelResults of the last run (for the harness)

_NC_CACHE = None


def _split_multi_waits(nc, templates):
    """Rewrite any instruction carrying >1 sem waits: keep the last wait,
    move each extra onto a fresh single-wait clone of the same-engine no-op
    template inserted immediately before it (engine streams are in-order)."""
    n = 0
    for f in nc.m.functions:
        for bb in f.blocks:
            newlist = []
            for ins in bb.instructions:
                si = getattr(ins, "sync_info", None)
                if si is not None and si.on_wait and len(si.on_wait) > 1:
                    extras = list(si.on_wait[:-1])
                    keep = list(si.on_wait[-1:])
                    tmpl = templates.get(ins.engine)
                    assert tmpl is not None, (
                        f"no wait-carrier template for engine {ins.engine} "
                        f"({type(ins).__name__} {ins.name})"
                    )
                    for w in extras:
                        c = copy.deepcopy(tmpl)
                        c.name = f"wcarrier-{n}"
                        n += 1
                        c.sync_info = mybir.SyncInfo(on_wait=[w], on_update=[])
                        newlist.append(c)
                    del si.on_wait[:]
                    si.on_wait.extend(keep)
                newlist.append(ins)
            bb.instructions[:] = newlist
    return n


def build():
    nc = bass.Bass()
    zt = nc.dram_tensor("zt", [D, R], BF16, kind="ExternalInput")
    out = nc.dram_tensor("partial", [1, 1], F32, kind="ExternalOutput")

    templates = {}

    with tile.TileContext(nc) as tc:
        with (
            tc.tile_pool(name="singles", bufs=1) as singles,
            tc.tile_pool(name="work", bufs=3) as work,
            tc.tile_pool(name="sbuf_s", bufs=3) as sbuf_s,
            tc.tile_pool(name="psum_g", bufs=3, space="PSUM") as psum_g,
            tc.tile_pool(name="psum_m", bufs=1, space="PSUM") as psum_m,
            tc.tile_pool(name="dram", bufs=1, space="DRAM") as dram,
        ):
            zt_k = [singles.tile([P, R], BF16, name=f"zt{k}") for k in range(KT)]
            Bt = singles.tile([P, R], BF16)            # inv-norm bcast by col
            I128 = singles.tile([P, P], F32)
            ones = singles.tile([P, 1], F32)
            ones_bf = singles.tile([P, 1], BF16)
            sumsq = singles.tile([P, CB], F32)
            normt = singles.tile([P, CB], F32)
            invt = singles.tile([P, CB], F32)
            slots = singles.tile([P, MT * NCH], F32)
            post = singles.tile([P, MT], F32)
            sumexp = singles.tile([P, MT], F32)
            lse = singles.tile([P, MT], F32)
            contribs = singles.tile([P, MT], F32)
            csum = singles.tile([P, 1], F32)
            out_sb = singles.tile([1, 1], F32)
            junk_exp = singles.tile([P, CW], F32)
            junk_pos = singles.tile([P, P], F32)
            trs = singles.tile([CB, P], F32)
            # wait-carrier scratches (one per engine, never read)
            scr_v = singles.tile([1, 1], F32)
            scr_a = singles.tile([1, 1], F32)
            scr_p = singles.tile([1, 1], F32)
            # shared misc PSUM bank: norm accum [128,64] / transpose [64,128]
            # / final [1,1] — used at disjoint times (Tile serializes).
            misc_ps = psum_m.tile([P, P], F32)
            dummy_ps = psum_m.tile([1, 1], F32)
            inv_dram = dram.tile([CB, P], F32)

            # --- wait-carrier templates (harmless one-off ops) ---
            c0 = nc.const_aps.tensor(0.0, (1, 1), F32)
            templates[mybir.EngineType.DVE] = nc.vector.memset(scr_v[:], 0).ins
            templates[mybir.EngineType.Activation] = nc.scalar.copy(
                scr_a[:], c0).ins
            templates[mybir.EngineType.Pool] = nc.gpsimd.memset(scr_p[:], 0).ins
            templates[mybir.EngineType.PE] = nc.tensor.matmul(
                dummy_ps[:], c0, c0, start=True, stop=True,
                skip_group_check=True).ins

            make_identity(nc, I128[:, :])
            nc.vector.memset(ones[:], 1.0)
            nc.vector.memset(ones_bf[:], 1.0)

            # Load resident Z^T (8 x 2 MB) and fold the norm reduction into
            # the stream: square each arriving chunk on DVE, reduce over the
            # k (partition) axis on PE with the squared chunk stationary.
            ss = misc_ps[:, 0:CB]                      # [128, 64] f32 accum
            for k in range(KT):
                nc.gpsimd.dma_start(out=zt_k[k][:], in_=zt[k * P:(k + 1) * P, :])
                for cc in range(R // CW):              # 8 x 1024-col chunks
                    sq = work.tile([P, CW], BF16, tag="sq")
                    nc.vector.tensor_mul(
                        sq[:], zt_k[k][:, cc * CW:(cc + 1) * CW],
                        zt_k[k][:, cc * CW:(cc + 1) * CW])
                    for cb in range(CW // P):          # 8 x 128-col blocks
                        g = cc * (CW // P) + cb
                        nc.tensor.matmul(
                            ss[:, g:g + 1],
                            sq[:, cb * P:(cb + 1) * P],
                            ones_bf[:],
                            start=(k == 0), stop=(k == KT - 1),
                            skip_group_check=True)
            nc.vector.tensor_copy(sumsq[:], ss)
            nc.scalar.sqrt(normt[:], sumsq[:])
            nc.vector.tensor_scalar_max(normt[:], normt[:], 1e-8)
            nc.vector.reciprocal(invt[:], normt[:])

            # Column-broadcast inv table: PE-transpose -> DRAM bounce ->
            # partition-broadcast DMA (bf16).
            trp = misc_ps[0:CB, :]                     # [64, 128] view
            nc.tensor.transpose(trp, invt[:], I128[:])
            nc.vector.tensor_copy(trs[:], trp)
            nc.gpsimd.dma_start(out=inv_dram[:], in_=trs[:])
            src = inv_dram[:]
            bcast = bass.AP(tensor=src.tensor, offset=src.offset,
                            ap=[[0, P], [1, R]])
            nc.gpsimd.dma_start(out=Bt[:], in_=bcast)

            inv_t = float(1.0 / TEMP)
            for j in range(NCH):
                for m in range(MT):
                    g = psum_g.tile([P, CW], F32)
                    for k in range(KT):
                        for h in range(CW // 512):  # N<=512 per matmul
                            nc.tensor.matmul(
                                g[:, h * 512:(h + 1) * 512],
                                zt_k[k][:, m * P:(m + 1) * P],
                                zt_k[k][:, j * CW + h * 512:j * CW + (h + 1) * 512],
                                start=(k == 0), stop=(k == KT - 1),
                                skip_group_check=True)
                    s = sbuf_s.tile([P, CW], F32)
                    nc.vector.scalar_tensor_tensor(
                        out=s[:], in0=g[:], scalar=invt[:, m:m + 1],
                        in1=Bt[:, j * CW:(j + 1) * CW],
                        op0=ALU.mult, op1=ALU.mult)
                    if j == 0:
                        # self-similarity diagonal -> -inf (via -100 pre /T)
                        off = m * P
                        nc.vector.scalar_tensor_tensor(
                            out=s[:, off:off + P], in0=I128[:], scalar=-100.0,
                            in1=s[:, off:off + P], op0=ALU.mult, op1=ALU.add)
                    if j == 4096 // CW:
                        # positive pair: rotated column = row + 4096.
                        # post[m] = sum(S_slice * I) (pre-1/T; folded later)
                        off = m * P
                        nc.vector.scalar_tensor_tensor(
                            out=junk_pos[:], in0=s[:, off:off + P], scalar=1.0,
                            in1=I128[:], op0=ALU.mult, op1=ALU.mult,
                            accum_out=post[:, m:m + 1])
                    nc.scalar.activation(
                        out=junk_exp[:], in_=s[:], func=ACTF.Exp,
                        scale=inv_t,
                        accum_out=slots[:, m * NCH + j:m * NCH + j + 1])

            for m in range(MT):
                nc.vector.reduce_sum(
                    out=sumexp[:, m:m + 1],
                    in_=slots[:, m * NCH:(m + 1) * NCH], axis=AX.X)
            nc.scalar.activation(out=lse[:], in_=sumexp[:], func=ACTF.Ln)
            # contribs = lse - post/T  ==  (post * -1/T) + lse
            nc.vector.scalar_tensor_tensor(
                out=contribs[:], in0=post[:], scalar=-inv_t,
                in1=lse[:], op0=ALU.mult, op1=ALU.add)
            nc.vector.reduce_sum(out=csum[:], in_=contribs[:], axis=AX.X)
            fin = misc_ps[0:1, 0:1]
            nc.tensor.matmul(fin, ones[:], csum[:], start=True, stop=True,
                             skip_group_check=True)
            nc.vector.tensor_copy(out_sb[:], fin)
            nc.gpsimd.dma_start(out=out[:], in_=out_sb[:])

    _split_multi_waits(nc, templates)
    return nc


def kernel(z_i: np.ndarray, z_j: np.ndarray) -> np.ndarray:
    global _NC_CACHE, LAST_RESULTS
    z = np.concatenate([np.asarray(z_i), np.asarray(z_j)], axis=0)
    zb = z.astype(ml_dtypes.bfloat16)

    in_maps = []
    for c in range(8):
        zrot = np.roll(zb, -c * MY, axis=0)
        in_maps.append({"zt": np.ascontiguousarray(zrot.T)})

    if _NC_CACHE is None:
        _NC_CACHE = build()

    res = run_bass_kernel_spmd(
        _NC_CACHE, in_maps, core_ids=list(range(8)), trace=TRACE)
    LAST_RESULTS = res

    total = 0.0
    for c in range(8):
        total += float(res.results[c]["partial"][0, 0])
    return np.float32(total / R)



# revision 4
# speedup vs baseline: 4.0189x; 4.0189x over previous
"""NT-Xent contrastive loss (SimCLR) on 8 Trainium2 NeuronCores.

Strategy (v2: host-norm + symmetry + fp8 DoubleRow):
  - Host: z = concat(z_i, z_j) [8192, 1024], L2-normalize rows in f32,
    scale by S=16, quantize to fp8 e4m3. The cosine-sim matrix is then
    just G = q @ q.T (scaled by S^2), no on-device normalization.
  - Symmetry: sim is symmetric. In rotated coords (each core's 1024 rows
    at block 0), core c computes only column blocks 0..4 (5/8 of the
    matrix). Blocks 1-3 contribute BOTH row-side exp-sums (via ACT accum)
    and column-side exp-sums (transpose rows, via per-column reductions);
    block 4 row-side only (its transpose is block 4 of the peer core);
    block 0 row-side with the self-diagonal masked. Every ordered (r,c)
    pair of the full 8192x8192 matrix is covered exactly once.
  - PE runs fp8e4 DoubleRow matmuls (2 k-subtiles of 128 per pass,
    0.5 cycles/row = 4x bf16 MAC throughput). Contraction 1024 = 4
    DoubleRow groups accumulated in PSUM.
  - exp((G - diag_mask) / (S^2*T)) on ACT straight out of PSUM with
    fused per-row accumulation (row-side sums). Column-side: DVE
    accumulates sum_m E_m in bf16, then two ones-matmuls on PE reduce
    over the partition axis; result DMAs straight from PSUM to DRAM.
  - Host combines: per-row sumexp = own row-side + 3 column-side chunks
    from neighbor cores; loss = mean(log(sumexp) - pos/(S^2*T)).

This container's walrus build only accepts ONE semaphore wait per
instruction (and none on CTRL-encoded ones like Drain), while Tile freely
emits several. Two workarounds below: the TileContext epilogue drain's waits
are re-emitted on DVE memsets, and a post-pass splits any multi-wait
instruction by inserting single-wait no-op "carrier" clones (per-engine
templates) just before it on the same engine stream. The PE carrier is a
[128,1] LDWEIGHTS (harmless: every real matmul loads its own weights).
"""

import copy

import numpy as np
import ml_dtypes


def _install_tile_drain_patch():
    import concourse.tile as tile
    from concourse import mybir
    from concourse.vector_clock import ScopedClock

    if getattr(tile.TileContext, "_drain_patch_installed", False):
        return

    def _drain_and_barrier(self, tick_clock, wait_clock):
        nc = self.nc
        drain_inst = nc.sync.drain()
        wait_clock.add_sem_waits(
            drain_inst.ins, ScopedClock({None: tick_clock.global_clock})
        )
        waits = list(drain_inst.ins.sync_info.on_wait)
        drain_inst.ins.sync_info.on_wait.clear()

        if waits:
            scr = nc.const_aps.tensor(0.0, (1, 1), mybir.dt.float32)
            for w in waits:
                ms = nc.vector.memset(scr, 0)
                if ms.ins.sync_info is None:
                    ms.ins.sync_info = mybir.SyncInfo(on_wait=[], on_update=[])
                ms.ins.sync_info.on_wait.append(w)

        nc.all_engine_barrier()
        assert self.sems is not None
        popped = nc._tile_sem_poison_stack.pop()
        assert popped is self._sem_poison
        nc.clear_and_free_semaphores(list(self.sems.allocated().values()))
        nc.all_engine_barrier()

    tile.TileContext._drain_and_barrier = _drain_and_barrier
    tile.TileContext._drain_patch_installed = True


_install_tile_drain_patch()

import concourse.bass as bass
import concourse.tile as tile
from concourse import mybir
from concourse.bass_utils import run_bass_kernel_spmd
from concourse.masks import make_identity

P = 128
D = 1024
R = 8192          # 2N rows
MY = 1024         # rows per core (= block size)
NB = 5            # column blocks computed per core (symmetry: 0..4)
KT = 8            # 128-deep k-subtiles in D
DKT = 4           # DoubleRow groups (256-deep each)
MT = 8            # m-tiles per core
CW = 1024         # column chunk width (= one block)
TEMP = 0.07
S = 16.0          # fp8 pre-scale; G = S^2 * sim
INVT2 = float(1.0 / (S * S * TEMP))
FP8 = mybir.dt.float8e4
BF16 = mybir.dt.bfloat16
F32 = mybir.dt.float32
ALU = mybir.AluOpType
ACTF = mybir.ActivationFunctionType
DR = mybir.MatmulPerfMode.DoubleRow

TRACE = False          # set True externally (test harness) for NTFF profiling
LAST_RESULTS = None    # BassKernelResults of the last run (for the harness)

_NC_CACHE = None


def _split_multi_waits(nc, templates):
    """Rewrite any instruction carrying >1 sem waits: keep the last wait,
    move each extra onto a fresh single-wait clone of the same-engine no-op
    template inserted immediately before it (engine streams are in-order)."""
    n = 0
    for f in nc.m.functions:
        for bb in f.blocks:
            newlist = []
            for ins in bb.instructions:
                si = getattr(ins, "sync_info", None)
                if si is not None and si.on_wait and len(si.on_wait) > 1:
                    extras = list(si.on_wait[:-1])
                    keep = list(si.on_wait[-1:])
                    tmpl = templates.get(ins.engine)
                    assert tmpl is not None, (
                        f"no wait-carrier template for engine {ins.engine} "
                        f"({type(ins).__name__} {ins.name})"
                    )
                    for w in extras:
                        c = copy.deepcopy(tmpl)
                        c.name = f"wcarrier-{n}"
                        n += 1
                        c.sync_info = mybir.SyncInfo(on_wait=[w], on_update=[])
                        newlist.append(c)
                    del si.on_wait[:]
                    si.on_wait.extend(keep)
                newlist.append(ins)
            bb.instructions[:] = newlist
    return n


def build():
    nc = bass.Bass()
    # [jc][p][ks][col]: element (p, ks, col) of chunk jc = q_rot[jc*CW+col,
    # ks*128+p]; flattened to rows jc*128+p, free ks*CW+col.
    zt = nc.dram_tensor("zt", [NB * P, KT * CW], FP8, kind="ExternalInput")
    slots_d = nc.dram_tensor("slots", [P, MT * NB], F32, kind="ExternalOutput")
    pos_d = nc.dram_tensor("pos", [P, MT], F32, kind="ExternalOutput")
    col_d = nc.dram_tensor("colsum", [3, CW], F32, kind="ExternalOutput")

    templates = {}

    with tile.TileContext(nc) as tc:
        with (
            tc.tile_pool(name="singles", bufs=1) as singles,
            tc.tile_pool(name="epool", bufs=3) as epool,
            tc.tile_pool(name="psum_g", bufs=3, space="PSUM") as psum_g,
            tc.tile_pool(name="psum_c", bufs=1, space="PSUM") as psum_c,
        ):
            zt_sb = [singles.tile([P, KT, CW], FP8, name=f"zt{j}")
                     for j in range(NB)]
            I128 = singles.tile([P, P], F32)
            ones_bf = singles.tile([P, 1], BF16)
            slots = singles.tile([P, MT * NB], F32)
            pos = singles.tile([P, MT], F32)
            junk_pos = singles.tile([P, P], F32)
            acc = [singles.tile([P, CW], BF16, name=f"acc{b}")
                   for b in range(1, 4)]
            colsb = [singles.tile([1, CW], F32, name=f"colsb{b}")
                     for b in range(1, 4)]
            # wait-carrier scratches (one per engine, never read)
            scr_v = singles.tile([1, 1], F32)
            scr_a = singles.tile([1, 1], F32)
            scr_p = singles.tile([1, 1], F32)

            # --- wait-carrier templates (harmless one-off ops) ---
            c0 = nc.const_aps.tensor(0.0, (1, 1), F32)
            templates[mybir.EngineType.DVE] = nc.vector.memset(scr_v[:], 0).ins
            templates[mybir.EngineType.Activation] = nc.scalar.copy(
                scr_a[:], c0).ins
            templates[mybir.EngineType.Pool] = nc.gpsimd.memset(scr_p[:], 0).ins
            templates[mybir.EngineType.PE] = nc.tensor.ldweights(
                ones_bf[:]).ins

            make_identity(nc, I128[:, :])
            nc.vector.memset(ones_bf[:], 1.0)

            for j in range(NB):
                nc.gpsimd.dma_start(
                    out=zt_sb[j][:, :, :],
                    in_=zt[j * P:(j + 1) * P, :])

            for jc in range(NB):
                for m in range(MT):
                    g = psum_g.tile([P, CW], F32, tag="g")
                    for dk in range(DKT):
                        lhsT = zt_sb[0][:, 2 * dk:2 * dk + 2,
                                        m * P:(m + 1) * P]
                        for h in range(CW // 512):
                            nc.tensor.matmul(
                                g[:, h * 512:(h + 1) * 512],
                                lhsT,
                                zt_sb[jc][:, 2 * dk:2 * dk + 2,
                                          h * 512:(h + 1) * 512],
                                start=(dk == 0), stop=(dk == DKT - 1),
                                perf_mode=DR,
                                skip_group_check=True)
                    if jc == 0:
                        # self-similarity diagonal -> big negative pre-scale
                        off = m * P
                        nc.vector.scalar_tensor_tensor(
                            out=g[:, off:off + P], in0=I128[:], scalar=-2000.0,
                            in1=g[:, off:off + P], op0=ALU.mult, op1=ALU.add)
                    if jc == 4:
                        # positive pair: rotated column = row + 4096
                        off = m * P
                        nc.vector.scalar_tensor_tensor(
                            out=junk_pos[:], in0=g[:, off:off + P], scalar=1.0,
                            in1=I128[:], op0=ALU.mult, op1=ALU.mult,
                            accum_out=pos[:, m:m + 1])
                    e = epool.tile([P, CW], BF16, tag="e")
                    nc.scalar.activation(
                        out=e[:], in_=g[:], func=ACTF.Exp, scale=INVT2,
                        accum_out=slots[:, m * NB + jc:m * NB + jc + 1])
                    if 1 <= jc <= 3:
                        # column-side partial: acc_b += E_m (bf16)
                        if m == 0:
                            nc.vector.tensor_copy(acc[jc - 1][:], e[:])
                        else:
                            nc.vector.tensor_tensor(
                                out=acc[jc - 1][:], in0=acc[jc - 1][:],
                                in1=e[:], op=ALU.add)
                if 1 <= jc <= 3:
                    # reduce acc over the partition axis: ones-matmul
                    cps = psum_c.tile([1, CW], F32, tag="cps")
                    for h in range(CW // 512):
                        nc.tensor.matmul(
                            cps[0:1, h * 512:(h + 1) * 512],
                            ones_bf[:],
                            acc[jc - 1][:, h * 512:(h + 1) * 512],
                            start=True, stop=True,
                            skip_group_check=True)
                    nc.vector.tensor_copy(colsb[jc - 1][:], cps[0:1, :])
                    nc.gpsimd.dma_start(
                        out=col_d[jc - 1:jc, :], in_=colsb[jc - 1][:])

            nc.gpsimd.dma_start(out=slots_d[:, :], in_=slots[:])
            nc.gpsimd.dma_start(out=pos_d[:, :], in_=pos[:])

    _split_multi_waits(nc, templates)
    return nc


def _prep_core_input(q8, c):
    """q8: [8192, 1024] fp8 (normalized*S). Returns the [640, 8192] fp8
    array for core c: rotated rows (own block first), first 5 blocks,
    k-subtile-major layout."""
    zr = np.roll(q8, -c * MY, axis=0)[:NB * MY]          # [5120, 1024]
    # chunk jc: [1024 cols][8 ks][128 p] -> [128 p][8 ks][1024 cols]
    a = zr.reshape(NB, CW, KT, P).transpose(0, 3, 2, 1)  # [5, 128, 8, 1024]
    return np.ascontiguousarray(a.reshape(NB * P, KT * CW))


def kernel(z_i: np.ndarray, z_j: np.ndarray) -> np.ndarray:
    global _NC_CACHE, LAST_RESULTS
    z = np.concatenate([np.asarray(z_i, dtype=np.float32),
                        np.asarray(z_j, dtype=np.float32)], axis=0)
    norm = np.maximum(np.sqrt((z.astype(np.float64) ** 2).sum(axis=1,
                                                              keepdims=True)),
                      1e-8)
    q8 = ((z / norm) * S).astype(ml_dtypes.float8_e4m3)

    in_maps = [{"zt": _prep_core_input(q8, c)} for c in range(8)]

    if _NC_CACHE is None:
        _NC_CACHE = build()

    res = run_bass_kernel_spmd(
        _NC_CACHE, in_maps, core_ids=list(range(8)), trace=TRACE)
    LAST_RESULTS = res

    sumexp = np.zeros(R, np.float64)
    pos_g = np.zeros(R, np.float64)
    for c in range(8):
        slots = res.results[c]["slots"].astype(np.float64)   # [128, m*5+jc]
        rs = slots.reshape(P, MT, NB).sum(axis=2)            # [p, m]
        sumexp[c * MY:(c + 1) * MY] += rs.T.reshape(MY)      # row i = m*128+p
        posv = res.results[c]["pos"].astype(np.float64)      # [p, m]
        pos_g[c * MY:(c + 1) * MY] = posv.T.reshape(MY) * INVT2
        col = res.results[c]["colsum"].astype(np.float64)    # [3, 1024]
        for b in (1, 2, 3):
            gb = (c + b) % 8
            sumexp[gb * MY:(gb + 1) * MY] += col[b - 1]
    loss = np.mean(np.log(sumexp) - pos_g)
    return np.float32(loss)
